# revision 13
# baseline (speedup 1.0000x reference)
"""GammaScorer edge-scoring kernel for 8 Trainium2 NeuronCores.

score[e] = sigmoid((x[src[e]] * x[dst[e]]) @ W.T + b)

Strategy: shard the 640K edges across 8 cores (80K each); replicate the
node table x (cast to fp16) and the tiny linear weights on every core.

Per core the gathers use the SWDGE `dma_gather` instruction (int16
indices, thousands of rows per instruction, so the ~1us Pool-engine
descriptor-generation fixed cost is amortized to noise). int16 can only
address 32K rows, so the node table is split into 4 chunks of 25000
rows and edges are bucketed host-side into 16 groups by (src_chunk,
dst_chunk). Edges of each global group are dealt round-robin across the
8 cores, so every core sees ~1/8 of every group and a single static
group capacity of 5120 slots (40 columns) covers all of them with ~1%
padding (dummy index-0 edges). Gathered slot i of a group lands at
SBUF [i%128, i//128, :].

Compute per group on the gathered [128, cols, 128] fp16 tiles: u = s*t
and v = u*w on the vector engine (fp16 2x perf mode), feature-dim
reduction via an fp16 add-tree + final fp32 reduce, bias+sigmoid on the
scalar engine. The host scatters the per-slot scores back to the
original edge order. The last group is gathered/computed in three
shrinking jobs (20/12/8 cols) so the post-last-gather tail is short.
"""

import sys

import numpy as np

sys.path.insert(0, "/opt/trn_rl_repo")

N_NODES = 100000
D = 128
E = 640000
N_CORES = 8
P = 128
NCHUNK = 4                  # node-table chunks (int16-addressable)
CHUNK = N_NODES // NCHUNK   # 25000 rows per chunk
NG = NCHUNK * NCHUNK        # 16 (src_chunk, dst_chunk) groups
GC = 40                     # gathered columns per group
G = GC * P                  # 5120 slots per group (>= max observed 5049)
SL = G // 16                # 320 idx slots per partition row
BUFS = 4                    # gather-tile ring size per side
LA = 3                      # gather lookahead (jobs)

_NC_CACHE = {}


def _build_nc():
    if "nc" in _NC_CACHE:
        return _NC_CACHE["nc"]

    from contextlib import ExitStack

    import concourse.bacc as bacc
    import concourse.tile as tile
    from concourse import mybir

    f32 = mybir.dt.float32
    f16 = mybir.dt.float16
    i16 = mybir.dt.int16

    nc = bacc.Bacc(
        "TRN2",
        target_bir_lowering=False,
        debug=False,
        num_devices=N_CORES,
    )
    x = nc.dram_tensor("x", [N_NODES, D], f16, kind="ExternalInput")
    hidx = nc.dram_tensor("hidx", [P, 2 * SL], i16, kind="ExternalInput")
    sidx = nc.dram_tensor("sidx", [P, NG * SL], i16, kind="ExternalInput")
    didx = nc.dram_tensor("didx", [P, NG * SL], i16, kind="ExternalInput")
    wrep = nc.dram_tensor("wrep", [P, D], f16, kind="ExternalInput")
    brep = nc.dram_tensor("brep", [P, 1], f32, kind="ExternalInput")
    out = nc.dram_tensor("out", [P, NG * GC], f32, kind="ExternalOutput")

    with tile.TileContext(nc) as tc, ExitStack() as ctx:
        const = ctx.enter_context(tc.tile_pool(name="const", bufs=1))
        gat = ctx.enter_context(tc.tile_pool(name="gat", bufs=BUFS))
        work = ctx.enter_context(tc.tile_pool(name="work", bufs=2))
        res = ctx.enter_context(tc.tile_pool(name="res", bufs=1))

        # group 0's s+d idx tables first, in one DMA, so gathers start fast
        hidx_sb = const.tile([P, 2 * SL], i16)
        nc.sync.dma_start(hidx_sb[:], hidx[:])
        sidx_sb = const.tile([P, NG * SL], i16)
        didx_sb = const.tile([P, NG * SL], i16)
        head = SL
        nc.sync.dma_start(sidx_sb[:, head:], sidx[:, head:])
        nc.sync.dma_start(didx_sb[:, head:], didx[:, head:])
        w_sb = const.tile([P, D], f16)
        nc.sync.dma_start(w_sb[:], wrep[:])
        b_sb = const.tile([P, 1], f32)
        nc.sync.dma_start(b_sb[:], brep[:])

        scores = res.tile([P, NG * GC], f32)

        tiles = {}

        # pipeline jobs: (group, col0, ncols); the last group is split in
        # shrinking pieces so the post-last-gather compute tail is short.
        jobs = [(g, 0, GC) for g in range(NG - 1)]
        jobs += [(NG - 1, 0, 32), (NG - 1, 32, 8)]

        def gathers(j):
            g, col0, ncols = jobs[j]
            a, b = g // NCHUNK, g % NCHUNK
            n = ncols * P
            sl0 = g * SL + col0 * P // 16
            s_t = gat.tile([P, ncols * D], f16, tag="S", name=f"s{j}")
            t_t = gat.tile([P, ncols * D], f16, tag="T", name=f"t{j}")
            tiles[j] = (s_t, t_t)
            if g == 0:
                # group 0's tables live in the fast-loading merged head
                c16 = col0 * P // 16
                s_ap = hidx_sb[:, c16 : c16 + n // 16]
                d_ap = hidx_sb[:, SL + c16 : SL + c16 + n // 16]
            else:
                s_ap = sidx_sb[:, sl0 : sl0 + n // 16]
                d_ap = didx_sb[:, sl0 : sl0 + n // 16]
            nc.gpsimd.dma_gather(
                s_t[:].rearrange("p (c d) -> p c d", d=D),
                x[a * CHUNK : (a + 1) * CHUNK, :],
                s_ap,
                n,
                n,
                D,
                single_packet=False,
            )
            nc.gpsimd.dma_gather(
                t_t[:].rearrange("p (c d) -> p c d", d=D),
                x[b * CHUNK : (b + 1) * CHUNK, :],
                d_ap,
                n,
                n,
                D,
                single_packet=False,
            )

        def compute_group(j):
            g, col0, ncols = jobs[j]
            s_t, t_t = tiles.pop(j)
            u = work.tile([P, ncols * D], f16, tag="U")
            nc.vector.tensor_mul(u[:], s_t[:], t_t[:])
            v = work.tile([P, ncols * D], f16, tag="V")
            nc.vector.tensor_tensor(
                out=v[:].rearrange("p (k d) -> p k d", d=D),
                in0=u[:].rearrange("p (k d) -> p k d", d=D),
                in1=w_sb[:].rearrange("p (o d) -> p o d", o=1).to_broadcast([P, ncols, D]),
                op=mybir.AluOpType.mult,
            )
            v3 = v[:].rearrange("p (k d) -> p k d", d=D)
            l1 = work.tile([P, ncols * 64], f16, tag="L1")
            nc.vector.tensor_add(
                l1[:].rearrange("p (k d) -> p k d", d=64),
                v3[:, :, 0:64],
                v3[:, :, 64:128],
            )
            l13 = l1[:].rearrange("p (k d) -> p k d", d=64)
            l2 = work.tile([P, ncols * 32], f16, tag="L2")
            nc.vector.tensor_add(
                l2[:].rearrange("p (k d) -> p k d", d=32),
                l13[:, :, 0:32],
                l13[:, :, 32:64],
            )
            l23 = l2[:].rearrange("p (k d) -> p k d", d=32)
            l3 = work.tile([P, ncols * 16], f16, tag="L3")
            nc.vector.tensor_add(
                l3[:].rearrange("p (k d) -> p k d", d=16),
                l23[:, :, 0:16],
                l23[:, :, 16:32],
            )
            dots = work.tile([P, ncols], f32, tag="dots")
            nc.vector.reduce_sum(
                dots[:],
                l3[:].rearrange("p (k d) -> p k d", d=16),
                axis=mybir.AxisListType.X,
            )
            c0 = g * GC + col0
            nc.scalar.activation(
                scores[:, c0 : c0 + ncols],
                dots[:],
                mybir.ActivationFunctionType.Sigmoid,
                bias=b_sb[:],
            )

        NJ = len(jobs)
        mid = NG - 3  # store the first 13 groups' scores early
        for j in range(min(LA, NJ)):
            gathers(j)
        for j in range(NJ):
            if j + LA < NJ:
                gathers(j + LA)
            compute_group(j)
            if j == mid - 1:
                nc.sync.dma_start(out[:, : mid * GC], scores[:, : mid * GC])

        nc.sync.dma_start(out[:, mid * GC :], scores[:, mid * GC :])

    nc.compile()
    _NC_CACHE["nc"] = nc
    return nc


def _wrap_idx(local_idx):
    """[NG, G] local row ids -> [P, NG*SL] int16 wrapped tables.

    Slot j of a group maps to partition j%16, column j//16; the 16-row
    block is replicated down all 128 partitions.
    """
    t = local_idx.reshape(NG, SL, 16).transpose(0, 2, 1)  # [NG, 16, SL]
    t = np.concatenate(list(t), axis=1)                   # [16, NG*SL]
    return np.ascontiguousarray(np.tile(t, (8, 1)))       # [128, NG*SL]


def _prep_in_maps(x, src_idx, dst_idx, W, b):
    x16 = np.ascontiguousarray(np.asarray(x), dtype=np.float16)
    src_idx = np.asarray(src_idx).astype(np.int64)
    dst_idx = np.asarray(dst_idx).astype(np.int64)
    W = np.asarray(W, dtype=np.float32)
    b = np.asarray(b, dtype=np.float32)

    wrep = np.ascontiguousarray(
        np.tile(W.reshape(1, D).astype(np.float16), (P, 1))
    )
    brep = np.full((P, 1), b.reshape(-1)[0], dtype=np.float32)

    # bucket all edges by (src_chunk, dst_chunk); deal each global group
    # round-robin across the 8 cores so per-core group sizes are ~equal
    grp = (src_idx // CHUNK) * NCHUNK + (dst_idx // CHUNK)
    order = np.argsort(grp, kind="stable")
    counts = np.bincount(grp, minlength=NG)
    assert counts.max() <= N_CORES * G, f"group overflow: {counts.max()}"
    offs = np.zeros(NG + 1, dtype=np.int64)
    np.cumsum(counts, out=offs[1:])

    slocal = np.zeros((N_CORES, NG, G), dtype=np.int16)
    dlocal = np.zeros((N_CORES, NG, G), dtype=np.int16)
    slot_to_edge = np.full((N_CORES, NG, G), -1, dtype=np.int64)
    for g in range(NG):
        eg = order[offs[g] : offs[g + 1]]
        for c in range(N_CORES):
            ecg = eg[c::N_CORES]
            n = len(ecg)
            assert n <= G
            slocal[c, g, :n] = (src_idx[ecg] % CHUNK).astype(np.int16)
            dlocal[c, g, :n] = (dst_idx[ecg] % CHUNK).astype(np.int16)
            slot_to_edge[c, g, :n] = ecg

    in_maps = []
    for c in range(N_CORES):
        ws = _wrap_idx(slocal[c])
        wd = _wrap_idx(dlocal[c])
        in_maps.append(
            {
                "x": x16,
                "hidx": np.ascontiguousarray(
                    np.concatenate([ws[:, :SL], wd[:, :SL]], axis=1)
                ),
                "sidx": ws,
                "didx": wd,
                "wrep": wrep,
                "brep": brep,
            }
        )
    return in_maps, slot_to_edge


_last_in_maps = None


def kernel(x, src_idx, dst_idx, W, b):
    from concourse.bass_utils import run_bass_kernel_spmd

    nc = _build_nc()
    in_maps, slot_to_edge = _prep_in_maps(x, src_idx, dst_idx, W, b)

    global _last_in_maps
    _last_in_maps = in_maps

    results = run_bass_kernel_spmd(nc, in_maps, list(range(N_CORES))).results

    out = np.empty(E, dtype=np.float32)
    for c in range(N_CORES):
        # [P, NG*GC] -> per group [P, GC]; slot i = col*128 + partition
        r = results[c]["out"].reshape(P, NG, GC)
        slotvals = r.transpose(1, 2, 0).reshape(NG, G)  # [g, col*128+p]
        s2e = slot_to_edge[c]
        valid = s2e >= 0
        out[s2e[valid]] = slotvals.reshape(NG, G)[valid]
    return out.reshape(E, 1)


# revision 14
# speedup vs baseline: 1.0117x; 1.0117x over previous
"""GammaScorer edge-scoring kernel for 8 Trainium2 NeuronCores.

score[e] = sigmoid((x[src[e]] * x[dst[e]]) @ W.T + b)

Strategy: shard the 640K edges across 8 cores (80K each); replicate the
node table x (cast to fp16) and the tiny linear weights on every core.

Per core the gathers use the SWDGE `dma_gather` instruction (int16
indices, thousands of rows per instruction, so the ~1us Pool-engine
descriptor-generation fixed cost is amortized to noise). int16 can only
address 32K rows, so the node table is split into 4 chunks of 25000
rows and edges are bucketed host-side into 16 groups by (src_chunk,
dst_chunk). Edges of each global group are dealt round-robin across the
8 cores, so every core sees ~1/8 of every group and a single static
group capacity of 5120 slots (40 columns) covers all of them with ~1%
padding (dummy index-0 edges). Gathered slot i of a group lands at
SBUF [i%128, i//128, :].

Compute per group on the gathered [128, cols, 128] fp16 tiles: u = s*t
and v = u*w on the vector engine (fp16 2x perf mode), feature-dim
reduction via an fp16 add-tree + final fp32 reduce, bias+sigmoid on the
scalar engine. The host scatters the per-slot scores back to the
original edge order. The last group is gathered/computed in three
shrinking jobs (20/12/8 cols) so the post-last-gather tail is short.
"""

import sys

import numpy as np

sys.path.insert(0, "/opt/trn_rl_repo")

N_NODES = 100000
D = 128
E = 640000
N_CORES = 8
P = 128
NCHUNK = 4                  # node-table chunks (int16-addressable)
CHUNK = N_NODES // NCHUNK   # 25000 rows per chunk
NG = NCHUNK * NCHUNK        # 16 (src_chunk, dst_chunk) groups
GC = 40                     # gathered columns per group
G = GC * P                  # 5120 slots per group (>= max observed 5049)
SL = G // 16                # 320 idx slots per partition row
BUFS = 4                    # gather-tile ring size per side
LA = 3                      # gather lookahead (jobs)

_NC_CACHE = {}


def _build_nc():
    if "nc" in _NC_CACHE:
        return _NC_CACHE["nc"]

    from contextlib import ExitStack

    import concourse.bacc as bacc
    import concourse.tile as tile
    from concourse import mybir

    f32 = mybir.dt.float32
    f16 = mybir.dt.float16
    i16 = mybir.dt.int16

    nc = bacc.Bacc(
        "TRN2",
        target_bir_lowering=False,
        debug=False,
        num_devices=N_CORES,
    )
    x = nc.dram_tensor("x", [N_NODES, D], f16, kind="ExternalInput")
    hidx = nc.dram_tensor("hidx", [P, 2 * SL], i16, kind="ExternalInput")
    sidx = nc.dram_tensor("sidx", [P, NG * SL], i16, kind="ExternalInput")
    didx = nc.dram_tensor("didx", [P, NG * SL], i16, kind="ExternalInput")
    wrep = nc.dram_tensor("wrep", [P, D], f16, kind="ExternalInput")
    brep = nc.dram_tensor("brep", [P, 1], f32, kind="ExternalInput")
    out = nc.dram_tensor("out", [P, NG * GC], f32, kind="ExternalOutput")

    with tile.TileContext(nc) as tc, ExitStack() as ctx:
        const = ctx.enter_context(tc.tile_pool(name="const", bufs=1))
        gat = ctx.enter_context(tc.tile_pool(name="gat", bufs=BUFS))
        work = ctx.enter_context(tc.tile_pool(name="work", bufs=2))
        res = ctx.enter_context(tc.tile_pool(name="res", bufs=1))

        # group 0's s+d idx tables first, in one DMA, so gathers start fast
        hidx_sb = const.tile([P, 2 * SL], i16)
        nc.sync.dma_start(hidx_sb[:], hidx[:])
        sidx_sb = const.tile([P, NG * SL], i16)
        didx_sb = const.tile([P, NG * SL], i16)
        head = SL
        nc.sync.dma_start(sidx_sb[:, head:], sidx[:, head:])
        nc.sync.dma_start(didx_sb[:, head:], didx[:, head:])
        w_sb = const.tile([P, D], f16)
        nc.sync.dma_start(w_sb[:], wrep[:])
        b_sb = const.tile([P, 1], f32)
        nc.sync.dma_start(b_sb[:], brep[:])

        scores = res.tile([P, NG * GC], f32)

        tiles = {}

        # pipeline jobs: (group, col0, ncols); the last group is split in
        # shrinking pieces so the post-last-gather compute tail is short.
        jobs = [(g, 0, GC) for g in range(NG - 1)]
        jobs += [(NG - 1, 0, 16), (NG - 1, 16, 12), (NG - 1, 28, 8), (NG - 1, 36, 4)]

        def gathers(j):
            g, col0, ncols = jobs[j]
            a, b = g // NCHUNK, g % NCHUNK
            n = ncols * P
            sl0 = g * SL + col0 * P // 16
            s_t = gat.tile([P, ncols * D], f16, tag="S", name=f"s{j}")
            t_t = gat.tile([P, ncols * D], f16, tag="T", name=f"t{j}")
            tiles[j] = (s_t, t_t)
            if g == 0:
                # group 0's tables live in the fast-loading merged head
                c16 = col0 * P // 16
                s_ap = hidx_sb[:, c16 : c16 + n // 16]
                d_ap = hidx_sb[:, SL + c16 : SL + c16 + n // 16]
            else:
                s_ap = sidx_sb[:, sl0 : sl0 + n // 16]
                d_ap = didx_sb[:, sl0 : sl0 + n // 16]
            nc.gpsimd.dma_gather(
                s_t[:].rearrange("p (c d) -> p c d", d=D),
                x[a * CHUNK : (a + 1) * CHUNK, :],
                s_ap,
                n,
                n,
                D,
                single_packet=False,
            )
            nc.gpsimd.dma_gather(
                t_t[:].rearrange("p (c d) -> p c d", d=D),
                x[b * CHUNK : (b + 1) * CHUNK, :],
                d_ap,
                n,
                n,
                D,
                single_packet=False,
            )

        def compute_group(j):
            g, col0, ncols = jobs[j]
            s_t, t_t = tiles.pop(j)
            u = work.tile([P, ncols * D], f16, tag="U")
            nc.vector.tensor_mul(u[:], s_t[:], t_t[:])
            v = work.tile([P, ncols * D], f16, tag="V")
            nc.vector.tensor_tensor(
                out=v[:].rearrange("p (k d) -> p k d", d=D),
                in0=u[:].rearrange("p (k d) -> p k d", d=D),
                in1=w_sb[:].rearrange("p (o d) -> p o d", o=1).to_broadcast([P, ncols, D]),
                op=mybir.AluOpType.mult,
            )
            v3 = v[:].rearrange("p (k d) -> p k d", d=D)
            l1 = work.tile([P, ncols * 64], f16, tag="L1")
            nc.vector.tensor_add(
                l1[:].rearrange("p (k d) -> p k d", d=64),
                v3[:, :, 0:64],
                v3[:, :, 64:128],
            )
            l13 = l1[:].rearrange("p (k d) -> p k d", d=64)
            l2 = work.tile([P, ncols * 32], f16, tag="L2")
            nc.vector.tensor_add(
                l2[:].rearrange("p (k d) -> p k d", d=32),
                l13[:, :, 0:32],
                l13[:, :, 32:64],
            )
            l23 = l2[:].rearrange("p (k d) -> p k d", d=32)
            l3 = work.tile([P, ncols * 16], f16, tag="L3")
            nc.vector.tensor_add(
                l3[:].rearrange("p (k d) -> p k d", d=16),
                l23[:, :, 0:16],
                l23[:, :, 16:32],
            )
            dots = work.tile([P, ncols], f32, tag="dots")
            nc.vector.reduce_sum(
                dots[:],
                l3[:].rearrange("p (k d) -> p k d", d=16),
                axis=mybir.AxisListType.X,
            )
            c0 = g * GC + col0
            nc.scalar.activation(
                scores[:, c0 : c0 + ncols],
                dots[:],
                mybir.ActivationFunctionType.Sigmoid,
                bias=b_sb[:],
            )

        NJ = len(jobs)
        mid = NG - 3  # store the first 13 groups' scores early
        for j in range(min(LA, NJ)):
            gathers(j)
        for j in range(NJ):
            if j + LA < NJ:
                gathers(j + LA)
            compute_group(j)
            if j == mid - 1:
                nc.sync.dma_start(out[:, : mid * GC], scores[:, : mid * GC])

        nc.sync.dma_start(out[:, mid * GC :], scores[:, mid * GC :])

    nc.compile()
    _NC_CACHE["nc"] = nc
    return nc


def _wrap_idx(local_idx):
    """[NG, G] local row ids -> [P, NG*SL] int16 wrapped tables.

    Slot j of a group maps to partition j%16, column j//16; the 16-row
    block is replicated down all 128 partitions.
    """
    t = local_idx.reshape(NG, SL, 16).transpose(0, 2, 1)  # [NG, 16, SL]
    t = np.concatenate(list(t), axis=1)                   # [16, NG*SL]
    return np.ascontiguousarray(np.tile(t, (8, 1)))       # [128, NG*SL]


def _prep_in_maps(x, src_idx, dst_idx, W, b):
    x16 = np.ascontiguousarray(np.asarray(x), dtype=np.float16)
    src_idx = np.asarray(src_idx).astype(np.int64)
    dst_idx = np.asarray(dst_idx).astype(np.int64)
    W = np.asarray(W, dtype=np.float32)
    b = np.asarray(b, dtype=np.float32)

    wrep = np.ascontiguousarray(
        np.tile(W.reshape(1, D).astype(np.float16), (P, 1))
    )
    brep = np.full((P, 1), b.reshape(-1)[0], dtype=np.float32)

    # bucket all edges by (src_chunk, dst_chunk); deal each global group
    # round-robin across the 8 cores so per-core group sizes are ~equal
    grp = (src_idx // CHUNK) * NCHUNK + (dst_idx // CHUNK)
    order = np.argsort(grp, kind="stable")
    counts = np.bincount(grp, minlength=NG)
    assert counts.max() <= N_CORES * G, f"group overflow: {counts.max()}"
    offs = np.zeros(NG + 1, dtype=np.int64)
    np.cumsum(counts, out=offs[1:])

    slocal = np.zeros((N_CORES, NG, G), dtype=np.int16)
    dlocal = np.zeros((N_CORES, NG, G), dtype=np.int16)
    slot_to_edge = np.full((N_CORES, NG, G), -1, dtype=np.int64)
    for g in range(NG):
        eg = order[offs[g] : offs[g + 1]]
        for c in range(N_CORES):
            ecg = eg[c::N_CORES]
            n = len(ecg)
            assert n <= G
            slocal[c, g, :n] = (src_idx[ecg] % CHUNK).astype(np.int16)
            dlocal[c, g, :n] = (dst_idx[ecg] % CHUNK).astype(np.int16)
            slot_to_edge[c, g, :n] = ecg

    in_maps = []
    for c in range(N_CORES):
        ws = _wrap_idx(slocal[c])
        wd = _wrap_idx(dlocal[c])
        in_maps.append(
            {
                "x": x16,
                "hidx": np.ascontiguousarray(
                    np.concatenate([ws[:, :SL], wd[:, :SL]], axis=1)
                ),
                "sidx": ws,
                "didx": wd,
                "wrep": wrep,
                "brep": brep,
            }
        )
    return in_maps, slot_to_edge


_last_in_maps = None


def kernel(x, src_idx, dst_idx, W, b):
    from concourse.bass_utils import run_bass_kernel_spmd

    nc = _build_nc()
    in_maps, slot_to_edge = _prep_in_maps(x, src_idx, dst_idx, W, b)

    global _last_in_maps
    _last_in_maps = in_maps

    results = run_bass_kernel_spmd(nc, in_maps, list(range(N_CORES))).results

    out = np.empty(E, dtype=np.float32)
    for c in range(N_CORES):
        # [P, NG*GC] -> per group [P, GC]; slot i = col*128 + partition
        r = results[c]["out"].reshape(P, NG, GC)
        slotvals = r.transpose(1, 2, 0).reshape(NG, G)  # [g, col*128+p]
        s2e = slot_to_edge[c]
        valid = s2e >= 0
        out[s2e[valid]] = slotvals.reshape(NG, G)[valid]
    return out.reshape(E, 1)
